# revision 12
# baseline (speedup 1.0000x reference)
"""AttentionBlock Trainium2 kernel (nn_AttentionBlock_74010876445388).

Strategy: data-parallel over batch (B=16 -> 2 per core x 8 cores).
Per core, per image, three engine-balanced phases:
  - PE window: GroupNorm (bn_stats + PE selector-matmul reduction),
    qkv projection with biases folded in via an augmented ones-row of h,
    v^T production, and the previous image's output projection. q/k are
    evicted from PSUM by the (otherwise idle) scalar engine.
  - ACT window: attention per head-pair with A/B half ping-pong so the
    scalar engine runs exp back-to-back: S^T = k^T q (f32r, row-group
    packed), exp straight out of PSUM into bf16, PV in bf16 (2 cols/cyc)
    with a ones-column accumulating the softmax denominator.
  - Softmax normalization via PE ones-broadcast of the denominators and
    a one-step Newton reciprocal on DVE (no slow RECIPROCAL/DIVIDE ops).
All heavy matmuls f32r or bf16; PE kept warm (no >3us idle gaps).
"""

import numpy as np

import concourse.bass as bass
import concourse.tile as tile
from concourse import bacc, mybir
from concourse.bass_utils import run_bass_kernel_spmd

N_CORES = 8
B, C, HW_L = 16, 512, 1024  # full batch, channels, flattened spatial
BPC = B // N_CORES  # batches per core = 2
NH = 8  # heads
CH = C // NH  # 64 channels/head
NG = 32  # groups
GS = C // NG  # 16 channels/group
L = HW_L
EPS = 1e-5
F32 = mybir.dt.float32
F32R = mybir.dt.float32r
BF16 = mybir.dt.bfloat16
I32 = mybir.dt.int32
AF = mybir.ActivationFunctionType
OP = mybir.AluOpType
RCP_MAGIC = 0x7EF311C3  # reciprocal bit-trick seed; 1 Newton iter -> 0.26% max err

_nc_cache = None


def _build():
    nc = bacc.Bacc("TRN2", target_bir_lowering=False)

    x2 = nc.dram_tensor("x2", [BPC, C, L], F32, kind="ExternalInput")
    # wqkT rows 0..511: channel-major weights; row 512: q/k biases
    wqkT = nc.dram_tensor("wqkT", [C + 1, 3 * C], F32, kind="ExternalInput")
    wpT = nc.dram_tensor("wpT", [C, C], F32, kind="ExternalInput")
    # packed per-partition constants: cols = beff[0:4] nw[4:8] nb[8:12]
    cvec = nc.dram_tensor("cvec", [128, 12], F32, kind="ExternalInput")
    sel = nc.dram_tensor("sel", [128, 4 * NG], F32, kind="ExternalInput")
    esel = nc.dram_tensor("esel", [NG, 4 * 128], F32, kind="ExternalInput")
    out_d = nc.dram_tensor("out", [BPC, C, L], F32, kind="ExternalOutput")

    from contextlib import ExitStack

    with tile.TileContext(nc) as tc:
        with ExitStack() as stack:
            ep = stack.enter_context
            cst_pool = ep(tc.tile_pool(name="const", bufs=1))
            wstage = ep(tc.tile_pool(name="wstage", bufs=1))
            xb_pool = ep(tc.tile_pool(name="xb", bufs=2))
            h_pool = ep(tc.tile_pool(name="hb", bufs=1))
            qk_pool = ep(tc.tile_pool(name="qk", bufs=1))
            vt_pool = ep(tc.tile_pool(name="vt", bufs=1))
            e_pool = ep(tc.tile_pool(name="ee", bufs=4))
            aun_pool = ep(tc.tile_pool(name="aun", bufs=2))
            nr_pool = ep(tc.tile_pool(name="nr", bufs=1))
            a_pool = ep(tc.tile_pool(name="ab", bufs=1))
            sm_pool = ep(tc.tile_pool(name="sm", bufs=2))
            o_pool = ep(tc.tile_pool(name="ob", bufs=2))
            ps_A = ep(tc.tile_pool(name="ps_A", bufs=1, space="PSUM"))
            ps_B = ep(tc.tile_pool(name="ps_B", bufs=1, space="PSUM"))
            ps_aA = ep(tc.tile_pool(name="ps_aA", bufs=1, space="PSUM"))
            ps_aB = ep(tc.tile_pool(name="ps_aB", bufs=1, space="PSUM"))
            # ---- persistent constants ----
            wq_sb = []
            for j in range(4):
                st = wstage.tile([128, 3 * C], F32, tag="wst")
                nc.gpsimd.dma_start(st, wqkT[128 * j : 128 * (j + 1), :])
                wt = cst_pool.tile([128, 3 * C], F32R, tag=f"wq{j}")
                nc.vector.tensor_copy(wt, st)
                wq_sb.append(wt)
            # bias row (row 512), f32r
            stg = wstage.tile([128, 3 * C], F32, tag="wst")
            stb = stg[0:1, :]
            nc.gpsimd.dma_start(stb, wqkT[C : C + 1, :])
            wqb_sb = cst_pool.tile([1, 3 * C], F32R, tag="wqb")
            nc.vector.tensor_copy(wqb_sb, stb)
            # proj weights in bf16
            wp_sb = []
            for j in range(4):
                stg = wstage.tile([128, 3 * C], F32, tag="wst")
                st = stg[:, 0:C]
                nc.gpsimd.dma_start(st, wpT[128 * j : 128 * (j + 1), :])
                wt = cst_pool.tile([128, C], BF16, tag=f"wp{j}")
                nc.vector.tensor_copy(wt, st)
                wp_sb.append(wt)
            stg = wstage.tile([128, 3 * C], F32, tag="wst")
            st = stg[:, 0 : 4 * NG]
            nc.sync.dma_start(st, sel.ap())
            sel_sb = cst_pool.tile([128, 4 * NG], F32R, tag="sel")
            nc.vector.tensor_copy(sel_sb, st)
            stg = wstage.tile([128, 3 * C], F32, tag="wst")
            st = stg[0:NG, 0 : 4 * 128]
            nc.sync.dma_start(st, esel.ap())
            esel_sb = cst_pool.tile([NG, 4 * 128], F32R, tag="esel")
            nc.vector.tensor_copy(esel_sb, st)
            cv = cst_pool.tile([128, 12], F32, tag="cvec")
            nc.sync.dma_start(cv, cvec.ap())
            beff_sb = [cv[:, i : i + 1] for i in range(0, 4)]
            nw_sb = [cv[:, i : i + 1] for i in range(4, 8)]
            nb_sb = [cv[:, i : i + 1] for i in range(8, 12)]
            ones_f = cst_pool.tile([128, 1], F32, tag="ones_f")
            nc.vector.memset(ones_f, 1.0)
            # ones row at partition 64 (matmul lhsT/rhs base partitions must match
            # the denominator row, which lives at partition 64 of a_un tiles)
            ones64 = cst_pool.tile([65, 64], F32R, tag="ones64")
            nc.vector.tensor_copy(
                ones64[64:65, :], ones_f[64:65, :].to_broadcast((1, 64))
            )
            ones1r = ones64[64:65, :]
            # augmented ones-row of h (for bias matmuls)
            h4 = cst_pool.tile([1, L], F32R, tag="h4")
            nc.vector.tensor_copy(h4, ones_f[0:1, :].to_broadcast((1, L)))

            # v^T tile persists across images; ones column written once
            vt_sb = vt_pool.tile([128, 8, 8, 65], BF16, tag="vt")
            nc.vector.tensor_copy(
                vt_sb[:, :, :, 64:65],
                ones_f[:, None, None, :].to_broadcast((128, 8, 8, 1)),
            )

            def wq_ap(j, p):
                return wq_sb[j][:, 128 * p : 128 * (p + 1)]

            def wk_ap(j, p):
                return wq_sb[j][:, 512 + 128 * p : 512 + 128 * (p + 1)]

            # prefetch x for both images up front (separate DMA queue)
            xt_b = []
            for b in range(BPC):
                xt = []
                for j in range(4):
                    x_t = xb_pool.tile([128, L], F32, tag=f"x{j}")
                    nc.sync.dma_start(x_t, x2[b, 128 * j : 128 * (j + 1), :])
                    xt.append(x_t)
                xt_b.append(xt)

            a_sb_prev = None
            for b in range(BPC):
                xt = xt_b[b]
                # ---- GroupNorm stats ----
                mv_t = []
                for j in range(4):
                    stats = sm_pool.tile([128, 2, 6], F32, tag="bnst")
                    nc.vector.bn_stats(stats[:, 0, :], xt[j][:, 0:512])
                    nc.vector.bn_stats(stats[:, 1, :], xt[j][:, 512:1024])
                    mv = sm_pool.tile([128, 2], F32, tag=f"mv{j}")
                    nc.vector.bn_aggr(mv, stats)
                    mv_t.append(mv)

                gps = ps_aA.tile([128, 1024], F32, tag="aA")
                for j in range(4):
                    ss = sm_pool.tile([128, 2], F32R, tag=f"ss{j}")
                    msq = sm_pool.tile([128, 1], F32, tag=f"msq{j}")
                    nc.vector.tensor_copy(ss[:, 0:1], mv_t[j][:, 0:1])
                    nc.vector.tensor_tensor(
                        msq, mv_t[j][:, 0:1], mv_t[j][:, 0:1], OP.mult
                    )
                    nc.vector.tensor_tensor(ss[:, 1:2], mv_t[j][:, 1:2], msq, OP.add)
                    nc.tensor.matmul(
                        gps[0:NG, 0:2],
                        sel_sb[:, NG * j : NG * (j + 1)],
                        ss,
                        start=(j == 0),
                        stop=(j == 3),
                    )

                gsb = sm_pool.tile([NG, 2], F32, tag="gsb")
                nc.vector.tensor_copy(gsb, gps[0:NG, 0:2])
                gm2 = sm_pool.tile([NG, 1], F32, tag="gm2")
                nc.vector.tensor_tensor(gm2, gsb[:, 0:1], gsb[:, 0:1], OP.mult)
                gvar = sm_pool.tile([NG, 1], F32, tag="gvar")
                nc.vector.tensor_tensor(gvar, gsb[:, 1:2], gm2, OP.subtract)
                nc.vector.tensor_scalar_add(gvar, gvar, EPS)
                # rsqrt via bit trick + 2 Newton iterations
                y_i = sm_pool.tile([NG, 1], I32, tag="rs_i")
                nc.vector.tensor_scalar(
                    y_i, gvar.bitcast(I32), 1, None, OP.logical_shift_right
                )
                nc.vector.tensor_scalar(y_i, y_i, -1, 0x5F3759DF, OP.mult, OP.add)
                y = y_i.bitcast(F32)
                gstats = sm_pool.tile([NG, 2], F32R, tag="gst")
                nc.vector.tensor_copy(gstats[:, 0:1], gsb[:, 0:1])
                tmp = sm_pool.tile([NG, 1], F32, tag="rs_t")
                for _ in range(2):
                    nc.vector.tensor_tensor(tmp, y, y, OP.mult)
                    nc.vector.tensor_tensor(tmp, tmp, gvar, OP.mult)
                    nc.vector.tensor_scalar(tmp, tmp, -0.5, 1.5, OP.mult, OP.add)
                    nc.vector.tensor_tensor(y, y, tmp, OP.mult)
                nc.vector.tensor_copy(gstats[:, 1:2], y)

                # expand per-group -> per-channel, normalize -> h (f32r)
                ht = []
                for j in range(4):
                    cst_ps = ps_aA.tile([128, 1024], F32, tag="aA")
                    nc.tensor.matmul(
                        cst_ps[:, 0:2],
                        esel_sb[:, 128 * j : 128 * (j + 1)],
                        gstats,
                        start=True,
                        stop=True,
                    )
                    sc = sm_pool.tile([128, 1], F32, tag=f"sc{j}")
                    nc.vector.tensor_tensor(sc, cst_ps[:, 1:2], nw_sb[j], OP.mult)
                    bi = sm_pool.tile([128, 1], F32, tag=f"bi{j}")
                    nc.vector.tensor_tensor(bi, cst_ps[:, 0:1], sc, OP.mult)
                    nc.vector.tensor_tensor(bi, nb_sb[j], bi, OP.subtract)
                    h_t = h_pool.tile([128, L], F32R, tag=f"h{j}")
                    nc.vector.tensor_scalar(h_t, xt[j], sc, bi, OP.mult, OP.add)
                    ht.append(h_t)

                # ---- qkv for all pairs (bias via augmented ones-row) ----
                qk_sb = []
                for p in range(4):
                    qps = ps_A.tile([128, 1024], F32, tag="A")
                    kps = ps_B.tile([128, 1024], F32, tag="B")
                    for n in range(2):
                        nsl = slice(512 * n, 512 * (n + 1))
                        for j in range(4):
                            nc.tensor.matmul(
                                qps[:, nsl],
                                wq_ap(j, p),
                                ht[j][:, nsl],
                                start=(j == 0),
                                stop=False,
                            )
                            nc.tensor.matmul(
                                kps[:, nsl],
                                wk_ap(j, p),
                                ht[j][:, nsl],
                                start=(j == 0),
                                stop=False,
                            )
                        nc.tensor.matmul(
                            qps[:, nsl],
                            wqb_sb[:, 128 * p : 128 * (p + 1)],
                            h4[:, nsl],
                            start=False,
                            stop=True,
                        )
                        nc.tensor.matmul(
                            kps[:, nsl],
                            wqb_sb[:, 512 + 128 * p : 512 + 128 * (p + 1)],
                            h4[:, nsl],
                            start=False,
                            stop=True,
                        )
                    # scalar-engine eviction (ACT otherwise idle here)
                    q_sb = qk_pool.tile([128, L], F32R, tag=f"q{p}")
                    nc.scalar.activation(q_sb, qps, AF.Copy)
                    k_sb = qk_pool.tile([128, L], F32R, tag=f"k{p}")
                    nc.scalar.activation(k_sb, kps, AF.Copy)
                    qk_sb.append((q_sb, k_sb))

                # ---- v^T production (bf16) ----
                for i in range(8):
                    vps = ps_aB.tile([128, 1024], F32, tag="aB")
                    for j in range(4):
                        nc.tensor.matmul(
                            vps[:, 0:512],
                            ht[j][:, 128 * i : 128 * (i + 1)],
                            wq_sb[j][:, 1024:1536],
                            start=(j == 0),
                            stop=(j == 3),
                        )
                    nc.vector.tensor_copy(
                        vt_sb[:, i, :, 0:64],
                        vps[:, 0:512].rearrange("p (h c) -> p h c", c=64),
                    )

                # ---- previous image's output projection (PE window) ----
                if a_sb_prev is not None:
                    for m in range(4):
                        pj = ps_aA.tile([128, 1024], F32, tag="aA")
                        for n in range(2):
                            nsl = slice(512 * n, 512 * (n + 1))
                            for j in range(4):
                                nc.tensor.matmul(
                                    pj[:, nsl],
                                    wp_sb[j][:, 128 * m : 128 * (m + 1)],
                                    a_sb_prev[j][:, nsl],
                                    start=(j == 0),
                                    stop=(j == 3),
                                )
                        o_t = o_pool.tile([128, L], F32, tag="o")
                        nc.scalar.activation(o_t, pj, AF.Identity, bias=beff_sb[m])
                        nc.sync.dma_start(
                            out_d[b - 1, 128 * m : 128 * (m + 1), :], o_t
                        )

                # ---- attention (ACT-bound window) ----
                a_sb = []
                for p in range(4):
                    q_sb, k_sb = qk_sb[p]
                    a_psA = ps_aA.tile([128, 1024], F32, tag="aA")
                    a_psB = ps_aB.tile([128, 1024], F32, tag="aB")
                    for sj in range(8):
                        sl = slice(128 * sj, 128 * (sj + 1))
                        for half, (s_pool, stag, base, a_ps) in enumerate(
                            ((ps_A, "A", 0, a_psA), (ps_B, "B", 64, a_psB))
                        ):
                            s_ps = s_pool.tile([128, 1024], F32, tag=stag)
                            for n in range(2):
                                nsl = slice(512 * n, 512 * (n + 1))
                                nc.tensor.matmul(
                                    s_ps[:, nsl],
                                    k_sb[base : base + 64, sl],
                                    q_sb[base : base + 64, nsl],
                                    start=True,
                                    stop=True,
                                    tile_position=(base, 0),
                                )
                            e_t = e_pool.tile([128, 1024], BF16, tag="e")
                            nc.scalar.activation(e_t, s_ps, AF.Exp, scale=0.125)
                            for n in range(2):
                                nsl = slice(512 * n, 512 * (n + 1))
                                nc.tensor.matmul(
                                    a_ps[0:65, nsl],
                                    vt_sb[:, sj, 2 * p + half, :],
                                    e_t[:, nsl],
                                    start=(sj == 0),
                                    stop=(sj == 7),
                                )
                    # ---- softmax normalization tail ----
                    a_unA = aun_pool.tile([65, 1024], F32R, tag="aunA")
                    nc.vector.tensor_copy(a_unA, a_psA[0:65, :])
                    a_unB = aun_pool.tile([65, 1024], F32R, tag="aunB")
                    nc.vector.tensor_copy(a_unB, a_psB[0:65, :])
                    # broadcast each half's denominator row into rows 0:64 of
                    # the accumulator slot it just vacated, then a one-step
                    # Newton reciprocal: r0 = bits(MAGIC - bits(d));
                    # negr = (d*r0 - 2)*r0 = -1/d (to 0.26%)
                    a_t = a_pool.tile([128, L], BF16, tag=f"a{p}")
                    for half, (a_un, ps_pool, ptag, ob) in enumerate(
                        ((a_unA, ps_aA, "aA", 0), (a_unB, ps_aB, "aB", 64))
                    ):
                        bc = ps_pool.tile([128, 1024], F32, tag=ptag)
                        for n in range(2):
                            nsl = slice(512 * n, 512 * (n + 1))
                            nc.tensor.matmul(
                                bc[0:64, nsl], ones1r, a_un[64:65, nsl],
                                start=True, stop=True,
                                tile_position=(64, 0),
                            )
                        r0i = nr_pool.tile([64, 1024], I32, tag="r0")
                        nc.vector.tensor_scalar(
                            r0i, bc[0:64, :].bitcast(I32), -1, RCP_MAGIC,
                            OP.mult, OP.add,
                        )
                        r0f = r0i.bitcast(F32)
                        t_nr = nr_pool.tile([64, 1024], F32, tag="t")
                        nc.vector.tensor_tensor(t_nr, bc[0:64, :], r0f, OP.mult)
                        negr = nr_pool.tile([64, 1024], F32, tag="negr")
                        nc.vector.scalar_tensor_tensor(
                            negr, t_nr, 2.0, r0f, OP.subtract, OP.mult
                        )
                        # a = a_un / den  (= (-a_un) * negr)
                        nc.vector.scalar_tensor_tensor(
                            a_t[ob : ob + 64, :], a_un[0:64, :], -1.0, negr,
                            OP.mult, OP.mult,
                        )
                    a_sb.append(a_t)
                a_sb_prev = a_sb

            # ---- final image's output projection ----
            for m in range(4):
                pj = ps_aA.tile([128, 1024], F32, tag="aA")
                for n in range(2):
                    nsl = slice(512 * n, 512 * (n + 1))
                    for j in range(4):
                        nc.tensor.matmul(
                            pj[:, nsl],
                            wp_sb[j][:, 128 * m : 128 * (m + 1)],
                            a_sb_prev[j][:, nsl],
                            start=(j == 0),
                            stop=(j == 3),
                        )
                o_t = o_pool.tile([128, L], F32, tag="o")
                nc.scalar.activation(o_t, pj, AF.Identity, bias=beff_sb[m])
                nc.sync.dma_start(out_d[BPC - 1, 128 * m : 128 * (m + 1), :], o_t)

    nc.compile()
    return nc


def _get_nc():
    global _nc_cache
    if _nc_cache is None:
        _nc_cache = _build()
    return _nc_cache


def _prep_inputs(x, norm_w, norm_b, w_qkv, b_qkv, w_proj, b_proj):
    x = np.asarray(x, dtype=np.float32).reshape(B, C, L)
    w_qkv = np.asarray(w_qkv, dtype=np.float32)
    b_qkv = np.asarray(b_qkv, dtype=np.float32)
    w_proj = np.asarray(w_proj, dtype=np.float32)
    b_proj = np.asarray(b_proj, dtype=np.float32)
    norm_w = np.asarray(norm_w, dtype=np.float32)
    norm_b = np.asarray(norm_b, dtype=np.float32)

    # column-reordered transposed qkv weight: [C+1, 3C] with
    # q head-major | k head-major | v head-major; row 512 = q/k biases
    wqkT = np.zeros((C + 1, 3 * C), dtype=np.float32)
    wT = w_qkv.T  # [C, 3C] original row order (per head: q,k,v)
    for h in range(NH):
        base = 192 * h
        wqkT[:C, 64 * h : 64 * (h + 1)] = wT[:, base : base + 64]
        wqkT[:C, 512 + 64 * h : 512 + 64 * (h + 1)] = wT[:, base + 64 : base + 128]
        wqkT[:C, 1024 + 64 * h : 1024 + 64 * (h + 1)] = wT[:, base + 128 : base + 192]
        wqkT[C, 64 * h : 64 * (h + 1)] = b_qkv[base : base + 64]
        wqkT[C, 512 + 64 * h : 512 + 64 * (h + 1)] = b_qkv[base + 64 : base + 128]
    wqkT = np.ascontiguousarray(wqkT)
    wpT = np.ascontiguousarray(w_proj.T)  # [C, C]

    # v bias folded into proj bias: b_eff = b_proj + w_proj @ bv
    bv = np.zeros((C,), dtype=np.float32)
    for h in range(NH):
        bv[64 * h : 64 * (h + 1)] = b_qkv[192 * h + 128 : 192 * h + 192]
    b_eff = (b_proj.astype(np.float64) + w_proj.astype(np.float64) @ bv).astype(
        np.float32
    )

    sel = np.zeros((128, 4 * NG), dtype=np.float32)
    esel = np.zeros((NG, 4 * 128), dtype=np.float32)
    for j in range(4):
        for c in range(128):
            sel[c, NG * j + 8 * j + c // GS] = 1.0 / GS
            esel[8 * j + c // GS, 128 * j + c] = 1.0

    cv = np.zeros((128, 12), dtype=np.float32)
    cv[:, 0:4] = b_eff.reshape(4, 128).T
    cv[:, 4:8] = norm_w.reshape(4, 128).T
    cv[:, 8:12] = norm_b.reshape(4, 128).T

    shared = {
        "wqkT": wqkT,
        "wpT": wpT,
        "cvec": cv,
        "sel": sel,
        "esel": esel,
    }
    in_maps = []
    for c in range(N_CORES):
        m = dict(shared)
        m["x2"] = np.ascontiguousarray(x[BPC * c : BPC * (c + 1)])
        in_maps.append(m)
    return in_maps


def _run(in_maps, trace=False):
    nc = _get_nc()
    return run_bass_kernel_spmd(
        nc, in_maps, core_ids=list(range(N_CORES)), trace=trace
    )


def kernel(x, norm_w, norm_b, w_qkv, b_qkv, w_proj, b_proj):
    in_maps = _prep_inputs(x, norm_w, norm_b, w_qkv, b_qkv, w_proj, b_proj)
    res = _run(in_maps)
    out = np.concatenate([r["out"] for r in res.results], axis=0)
    return out.astype(np.float32)


# revision 24
# speedup vs baseline: 1.3247x; 1.3247x over previous
"""AttentionBlock Trainium2 kernel (nn_AttentionBlock_74010876445388).

Strategy: data-parallel over batch (B=16 -> 2 per core x 8 cores).
Per core, per image, three engine-balanced phases:
  - PE window: GroupNorm (bn_stats + PE selector-matmul reduction),
    qkv projection with biases folded in via an augmented ones-row of h,
    v^T production, and the previous image's output projection. q/k are
    evicted from PSUM by the (otherwise idle) scalar engine.
  - ACT window: attention per head-pair with A/B half ping-pong so the
    scalar engine runs exp back-to-back: S^T = k^T q (f32r, row-group
    packed), exp straight out of PSUM into bf16, PV in bf16 (2 cols/cyc)
    with a ones-column accumulating the softmax denominator.
  - Softmax normalization via PE ones-broadcast of the denominators and
    a one-step Newton reciprocal on DVE (no slow RECIPROCAL/DIVIDE ops).
All heavy matmuls f32r or bf16; PE kept warm (no >3us idle gaps).
"""

import numpy as np

import concourse.bass as bass
import concourse.tile as tile
from concourse import bacc, mybir
from concourse.bass_utils import run_bass_kernel_spmd

N_CORES = 8
B, C, HW_L = 16, 512, 1024  # full batch, channels, flattened spatial
BPC = B // N_CORES  # batches per core = 2
NH = 8  # heads
CH = C // NH  # 64 channels/head
NG = 32  # groups
GS = C // NG  # 16 channels/group
L = HW_L
EPS = 1e-5
F32 = mybir.dt.float32
F32R = mybir.dt.float32r
BF16 = mybir.dt.bfloat16
I32 = mybir.dt.int32
AF = mybir.ActivationFunctionType
OP = mybir.AluOpType
RCP_MAGIC = 0x7EF311C3  # reciprocal bit-trick seed; 1 Newton iter -> 0.26% max err

_nc_cache = None


def _build():
    nc = bacc.Bacc("TRN2", target_bir_lowering=False)

    x2 = nc.dram_tensor("x2", [BPC, C, L], F32, kind="ExternalInput")
    # wqkT rows 0..511: channel-major weights; row 512: q/k biases
    wqkT = nc.dram_tensor("wqkT", [C + 1, 3 * C], F32, kind="ExternalInput")
    wpT = nc.dram_tensor("wpT", [C, C], F32, kind="ExternalInput")
    # packed per-partition constants: cols = beff[0:4] nw[4:8] nb[8:12]
    cvec = nc.dram_tensor("cvec", [128, 12], F32, kind="ExternalInput")
    sel = nc.dram_tensor("sel", [128, 4 * NG], F32, kind="ExternalInput")
    esel = nc.dram_tensor("esel", [NG, 4 * 128], F32, kind="ExternalInput")
    out_d = nc.dram_tensor("out", [BPC, C, L], F32, kind="ExternalOutput")

    from contextlib import ExitStack

    with tile.TileContext(nc) as tc:
        with ExitStack() as stack:
            ep = stack.enter_context
            cst_pool = ep(tc.tile_pool(name="const", bufs=1))
            wstage = ep(tc.tile_pool(name="wstage", bufs=2))
            xb_pool = ep(tc.tile_pool(name="xb", bufs=1))
            h_pool = ep(tc.tile_pool(name="hb", bufs=1))
            qk_pool = ep(tc.tile_pool(name="qk", bufs=1))
            vt_pool = ep(tc.tile_pool(name="vt", bufs=1))
            e_pool = ep(tc.tile_pool(name="ee", bufs=3))
            aun_pool = ep(tc.tile_pool(name="aun", bufs=2))
            nr_pool = ep(tc.tile_pool(name="nr", bufs=1))
            a_pool = ep(tc.tile_pool(name="ab", bufs=1))
            sm_pool = ep(tc.tile_pool(name="sm", bufs=2))
            o_pool = ep(tc.tile_pool(name="ob", bufs=1))
            ps_A = ep(tc.tile_pool(name="ps_A", bufs=1, space="PSUM"))
            ps_B = ep(tc.tile_pool(name="ps_B", bufs=1, space="PSUM"))
            ps_aA = ep(tc.tile_pool(name="ps_aA", bufs=1, space="PSUM"))
            ps_aB = ep(tc.tile_pool(name="ps_aB", bufs=1, space="PSUM"))
            # prefetch x for both images first (sync DMA queue, needed first)
            xt_b = []
            for b in range(BPC):
                xt = []
                for j in range(4):
                    x_t = xb_pool.tile([128, L], F32, tag=f"x{j}")
                    eng = nc.sync if j < 2 else nc.gpsimd
                    eng.dma_start(x_t, x2[b, 128 * j : 128 * (j + 1), :])
                    xt.append(x_t)
                xt_b.append(xt)

            # ---- persistent constants ----
            # f32r rounding must come from a real op; use the otherwise-idle
            # scalar/gpsimd engines so DVE stays free for GroupNorm at startup
            stg0 = wstage.tile([128, 4 * NG], F32, tag="selst")
            nc.sync.dma_start(stg0, sel.ap())
            sel_sb = cst_pool.tile([128, 4 * NG], F32R, tag="sel")
            nc.scalar.activation(sel_sb, stg0, AF.Copy)
            stg1 = wstage.tile([NG, 4 * 128], F32, tag="eselst")
            nc.sync.dma_start(stg1, esel.ap())
            esel_sb = cst_pool.tile([NG, 4 * 128], F32R, tag="esel")
            nc.scalar.activation(esel_sb, stg1, AF.Copy)
            wq_sb = []
            for j in range(4):
                st = wstage.tile([128, 3 * C], F32, tag="wst")
                nc.gpsimd.dma_start(st, wqkT[128 * j : 128 * (j + 1), :])
                wt = cst_pool.tile([128, 3 * C], F32R, tag=f"wq{j}")
                nc.scalar.activation(wt, st, AF.Copy)
                wq_sb.append(wt)
            stg2 = wstage.tile([128, 3 * C], F32, tag="wst")
            stb = stg2[0:1, :]
            nc.gpsimd.dma_start(stb, wqkT[C : C + 1, :])
            wqb_sb = cst_pool.tile([1, 3 * C], F32R, tag="wqb")
            nc.scalar.activation(wqb_sb, stb, AF.Copy)
            wp_sb = []
            for j in range(4):
                stg3 = wstage.tile([128, 3 * C], F32, tag="wst")
                st = stg3[:, 0:C]
                nc.gpsimd.dma_start(st, wpT[128 * j : 128 * (j + 1), :])
                wt = cst_pool.tile([128, C], F32R, tag=f"wp{j}")
                nc.gpsimd.tensor_copy(wt, st)
                wp_sb.append(wt)
            cv = cst_pool.tile([128, 12], F32, tag="cvec")
            nc.sync.dma_start(cv, cvec.ap())
            beff_sb = [cv[:, i : i + 1] for i in range(0, 4)]
            nw_sb = [cv[:, i : i + 1] for i in range(4, 8)]
            nb_sb = [cv[:, i : i + 1] for i in range(8, 12)]
            ones_f = cst_pool.tile([128, 1], F32, tag="ones_f")
            nc.vector.memset(ones_f, 1.0)
            # ones row at partition 64 (matmul lhsT/rhs base partitions must match
            # the denominator row, which lives at partition 64 of a_un tiles)
            ones64 = cst_pool.tile([65, 64], F32R, tag="ones64")
            nc.vector.tensor_copy(
                ones64[64:65, :], ones_f[64:65, :].to_broadcast((1, 64))
            )
            ones1r = ones64[64:65, :]
            # augmented ones-row of h (for bias matmuls)
            h4 = cst_pool.tile([1, L], F32R, tag="h4")
            nc.vector.tensor_copy(h4, ones_f[0:1, :].to_broadcast((1, L)))



            def wq_ap(j, p):
                return wq_sb[j][:, 128 * p : 128 * (p + 1)]

            def wk_ap(j, p):
                return wq_sb[j][:, 512 + 128 * p : 512 + 128 * (p + 1)]

            def group_norm(xt):
                # ---- GroupNorm stats -> normalized h tiles (f32r) ----
                mv_t = []
                for j in range(4):
                    stats = sm_pool.tile([128, 2, 6], F32, tag="bnst")
                    nc.vector.bn_stats(stats[:, 0, :], xt[j][:, 0:512])
                    nc.vector.bn_stats(stats[:, 1, :], xt[j][:, 512:1024])
                    mv = sm_pool.tile([128, 2], F32, tag=f"mv{j}")
                    nc.vector.bn_aggr(mv, stats)
                    mv_t.append(mv)

                gps = ps_aA.tile([128, 1024], F32, tag="aA")
                for j in range(4):
                    ss = sm_pool.tile([128, 2], F32R, tag=f"ss{j}")
                    msq = sm_pool.tile([128, 1], F32, tag=f"msq{j}")
                    nc.vector.tensor_copy(ss[:, 0:1], mv_t[j][:, 0:1])
                    nc.vector.tensor_tensor(
                        msq, mv_t[j][:, 0:1], mv_t[j][:, 0:1], OP.mult
                    )
                    nc.vector.tensor_tensor(ss[:, 1:2], mv_t[j][:, 1:2], msq, OP.add)
                    nc.tensor.matmul(
                        gps[0:NG, 0:2],
                        sel_sb[:, NG * j : NG * (j + 1)],
                        ss,
                        start=(j == 0),
                        stop=(j == 3),
                    )

                gsb = sm_pool.tile([NG, 2], F32, tag="gsb")
                nc.vector.tensor_copy(gsb, gps[0:NG, 0:2])
                gm2 = sm_pool.tile([NG, 1], F32, tag="gm2")
                nc.vector.tensor_tensor(gm2, gsb[:, 0:1], gsb[:, 0:1], OP.mult)
                gvar = sm_pool.tile([NG, 1], F32, tag="gvar")
                nc.vector.tensor_tensor(gvar, gsb[:, 1:2], gm2, OP.subtract)
                nc.vector.tensor_scalar_add(gvar, gvar, EPS)
                # rsqrt via bit trick + 2 Newton iterations
                y_i = sm_pool.tile([NG, 1], I32, tag="rs_i")
                nc.vector.tensor_scalar(
                    y_i, gvar.bitcast(I32), 1, None, OP.logical_shift_right
                )
                nc.vector.tensor_scalar(y_i, y_i, -1, 0x5F3759DF, OP.mult, OP.add)
                y = y_i.bitcast(F32)
                gstats = sm_pool.tile([NG, 2], F32R, tag="gst")
                nc.vector.tensor_copy(gstats[:, 0:1], gsb[:, 0:1])
                tmp = sm_pool.tile([NG, 1], F32, tag="rs_t")
                for _ in range(2):
                    nc.vector.tensor_tensor(tmp, y, y, OP.mult)
                    nc.vector.tensor_tensor(tmp, tmp, gvar, OP.mult)
                    nc.vector.tensor_scalar(tmp, tmp, -0.5, 1.5, OP.mult, OP.add)
                    nc.vector.tensor_tensor(y, y, tmp, OP.mult)
                nc.vector.tensor_copy(gstats[:, 1:2], y)

                # expand per-group -> per-channel, normalize -> h (f32r)
                ht = []
                for j in range(4):
                    cst_ps = ps_aA.tile([128, 1024], F32, tag="aA")
                    nc.tensor.matmul(
                        cst_ps[:, 0:2],
                        esel_sb[:, 128 * j : 128 * (j + 1)],
                        gstats,
                        start=True,
                        stop=True,
                    )
                    sc = sm_pool.tile([128, 1], F32, tag=f"sc{j}")
                    nc.vector.tensor_tensor(sc, cst_ps[:, 1:2], nw_sb[j], OP.mult)
                    bi = sm_pool.tile([128, 1], F32, tag=f"bi{j}")
                    nc.vector.tensor_tensor(bi, cst_ps[:, 0:1], sc, OP.mult)
                    nc.vector.tensor_tensor(bi, nb_sb[j], bi, OP.subtract)
                    h_t = h_pool.tile([128, L], F32R, tag=f"h{j}")
                    nc.vector.tensor_scalar(h_t, xt[j], sc, bi, OP.mult, OP.add)
                    ht.append(h_t)
                return ht

            def qkv_pairs(ht):
                # ---- qkv for all pairs (bias via augmented ones-row);
                # j-outer so consecutive matmuls share the stationary operand
                qk_sb = []
                for p in range(4):
                    qps = ps_A.tile([128, 1024], F32, tag="A")
                    kps = ps_B.tile([128, 1024], F32, tag="B")
                    for j in range(4):
                        for n in range(2):
                            nsl = slice(512 * n, 512 * (n + 1))
                            nc.tensor.matmul(
                                qps[:, nsl], wq_ap(j, p), ht[j][:, nsl],
                                start=(j == 0), stop=False,
                            )
                        for n in range(2):
                            nsl = slice(512 * n, 512 * (n + 1))
                            nc.tensor.matmul(
                                kps[:, nsl], wk_ap(j, p), ht[j][:, nsl],
                                start=(j == 0), stop=False,
                            )
                    for n in range(2):
                        nsl = slice(512 * n, 512 * (n + 1))
                        nc.tensor.matmul(
                            qps[:, nsl], wqb_sb[:, 128 * p : 128 * (p + 1)],
                            h4[:, nsl], start=False, stop=True,
                        )
                    for n in range(2):
                        nsl = slice(512 * n, 512 * (n + 1))
                        nc.tensor.matmul(
                            kps[:, nsl],
                            wqb_sb[:, 512 + 128 * p : 512 + 128 * (p + 1)],
                            h4[:, nsl], start=False, stop=True,
                        )
                    # scalar-engine eviction (ACT otherwise idle here)
                    q_sb = qk_pool.tile([128, L], F32R, tag=f"q{p}")
                    nc.scalar.activation(q_sb, qps, AF.Copy)
                    k_sb = qk_pool.tile([128, L], F32R, tag=f"k{p}")
                    nc.scalar.activation(k_sb, kps, AF.Copy)
                    qk_sb.append((q_sb, k_sb))
                return qk_sb

            def v_transpose(ht):
                vt_sb = vt_pool.tile([128, 8, 8, 65], F32R, tag="vt")
                nc.vector.tensor_copy(
                    vt_sb[:, :, :, 64:65],
                    ones_f[:, None, None, :].to_broadcast((128, 8, 8, 1)),
                )
                for i in range(8):
                    vps = ps_aB.tile([128, 1024], F32, tag="aB")
                    for j in range(4):
                        nc.tensor.matmul(
                            vps[:, 0:512],
                            ht[j][:, 128 * i : 128 * (i + 1)],
                            wq_sb[j][:, 1024:1536],
                            start=(j == 0),
                            stop=(j == 3),
                        )
                    nc.vector.tensor_copy(
                        vt_sb[:, i, :, 0:64],
                        vps[:, 0:512].rearrange("p (h c) -> p h c", c=64),
                    )
                return vt_sb

            def proj_out(a_sb, b):
                for m in range(4):
                    pj = ps_aA.tile([128, 1024], F32, tag="aA")
                    for j in range(4):
                        for n in range(2):
                            nsl = slice(512 * n, 512 * (n + 1))
                            nc.tensor.matmul(
                                pj[:, nsl],
                                wp_sb[j][:, 128 * m : 128 * (m + 1)],
                                a_sb[j][:, nsl],
                                start=(j == 0),
                                stop=(j == 3),
                            )
                    o_t = o_pool.tile([128, L], F32, tag="o")
                    nc.vector.tensor_scalar_add(o_t, pj, beff_sb[m])
                    nc.sync.dma_start(out_d[b, 128 * m : 128 * (m + 1), :], o_t)

            def attention(qk_sb, vt_sb):
                # ---- attention (ACT-bound window) ----
                a_sb = []
                for p in range(4):
                    q_sb, k_sb = qk_sb[p]
                    a_psA = ps_aA.tile([128, 1024], F32, tag="aA")
                    a_psB = ps_aB.tile([128, 1024], F32, tag="aB")
                    for sj in range(8):
                        sl = slice(128 * sj, 128 * (sj + 1))
                        for half, (s_pool, stag, base, a_ps) in enumerate(
                            ((ps_A, "A", 0, a_psA), (ps_B, "B", 64, a_psB))
                        ):
                            s_ps = s_pool.tile([128, 1024], F32, tag=stag)
                            for n in range(2):
                                nsl = slice(512 * n, 512 * (n + 1))
                                nc.tensor.matmul(
                                    s_ps[:, nsl],
                                    k_sb[base : base + 64, sl],
                                    q_sb[base : base + 64, nsl],
                                    start=True,
                                    stop=True,
                                    tile_position=(base, 0),
                                )
                            e_t = e_pool.tile([128, 1024], F32R, tag="e")
                            nc.scalar.activation(e_t, s_ps, AF.Exp, scale=0.125)
                            for n in range(2):
                                nsl = slice(512 * n, 512 * (n + 1))
                                nc.tensor.matmul(
                                    a_ps[0:65, nsl],
                                    vt_sb[:, sj, 2 * p + half, :],
                                    e_t[:, nsl],
                                    start=(sj == 0),
                                    stop=(sj == 7),
                                )
                    # ---- softmax normalization tail ----
                    a_unA = aun_pool.tile([65, 1024], F32R, tag="aunA")
                    nc.vector.tensor_copy(a_unA, a_psA[0:65, :])
                    a_unB = aun_pool.tile([65, 1024], F32R, tag="aunB")
                    nc.vector.tensor_copy(a_unB, a_psB[0:65, :])
                    # broadcast each half's denominator row into rows 0:64 of
                    # the accumulator slot it just vacated, then a one-step
                    # Newton reciprocal: r0 = bits(MAGIC - bits(d));
                    # negr = (d*r0 - 2)*r0 = -1/d (to 0.26%)
                    a_t = a_pool.tile([128, L], F32R, tag=f"a{p}")
                    for half, (a_un, ps_pool, ptag, ob) in enumerate(
                        ((a_unA, ps_aA, "aA", 0), (a_unB, ps_aB, "aB", 64))
                    ):
                        bc = ps_pool.tile([128, 1024], F32, tag=ptag)
                        for n in range(2):
                            nsl = slice(512 * n, 512 * (n + 1))
                            nc.tensor.matmul(
                                bc[0:64, nsl], ones1r, a_un[64:65, nsl],
                                start=True, stop=True,
                                tile_position=(64, 0),
                            )
                        r0i = nr_pool.tile([64, 1024], I32, tag="r0")
                        nc.vector.tensor_scalar(
                            r0i, bc[0:64, :].bitcast(I32), -1, RCP_MAGIC,
                            OP.mult, OP.add,
                        )
                        r0f = r0i.bitcast(F32)
                        t_nr = nr_pool.tile([64, 1024], F32, tag="t")
                        nc.vector.tensor_tensor(t_nr, bc[0:64, :], r0f, OP.mult)
                        negr = nr_pool.tile([64, 1024], F32, tag="negr")
                        nc.vector.scalar_tensor_tensor(
                            negr, t_nr, 2.0, r0f, OP.subtract, OP.mult
                        )
                        # a = a_un / den  (= (-a_un) * negr)
                        nc.vector.scalar_tensor_tensor(
                            a_t[ob : ob + 64, :], a_un[0:64, :], -1.0, negr,
                            OP.mult, OP.mult,
                        )
                    a_sb.append(a_t)
                return a_sb

            # ---- driver: image 1's GN and v^T are emitted before image 0's
            # attention so they fill PE slack inside the ACT-bound window
            ht0 = group_norm(xt_b[0])
            qk0 = qkv_pairs(ht0)
            vt0 = v_transpose(ht0)
            ht1 = group_norm(xt_b[1])
            a0 = attention(qk0, vt0)
            qk1 = qkv_pairs(ht1)
            vt1 = v_transpose(ht1)
            proj_out(a0, 0)
            a1 = attention(qk1, vt1)
            proj_out(a1, 1)

    nc.compile()
    return nc


def _get_nc():
    global _nc_cache
    if _nc_cache is None:
        _nc_cache = _build()
    return _nc_cache


def _prep_inputs(x, norm_w, norm_b, w_qkv, b_qkv, w_proj, b_proj):
    x = np.asarray(x, dtype=np.float32).reshape(B, C, L)
    w_qkv = np.asarray(w_qkv, dtype=np.float32)
    b_qkv = np.asarray(b_qkv, dtype=np.float32)
    w_proj = np.asarray(w_proj, dtype=np.float32)
    b_proj = np.asarray(b_proj, dtype=np.float32)
    norm_w = np.asarray(norm_w, dtype=np.float32)
    norm_b = np.asarray(norm_b, dtype=np.float32)

    # column-reordered transposed qkv weight: [C+1, 3C] with
    # q head-major | k head-major | v head-major; row 512 = q/k biases
    wqkT = np.zeros((C + 1, 3 * C), dtype=np.float32)
    wT = w_qkv.T  # [C, 3C] original row order (per head: q,k,v)
    for h in range(NH):
        base = 192 * h
        wqkT[:C, 64 * h : 64 * (h + 1)] = wT[:, base : base + 64]
        wqkT[:C, 512 + 64 * h : 512 + 64 * (h + 1)] = wT[:, base + 64 : base + 128]
        wqkT[:C, 1024 + 64 * h : 1024 + 64 * (h + 1)] = wT[:, base + 128 : base + 192]
        wqkT[C, 64 * h : 64 * (h + 1)] = b_qkv[base : base + 64]
        wqkT[C, 512 + 64 * h : 512 + 64 * (h + 1)] = b_qkv[base + 64 : base + 128]
    wqkT = np.ascontiguousarray(wqkT)
    wpT = np.ascontiguousarray(w_proj.T)  # [C, C]

    # v bias folded into proj bias: b_eff = b_proj + w_proj @ bv
    bv = np.zeros((C,), dtype=np.float32)
    for h in range(NH):
        bv[64 * h : 64 * (h + 1)] = b_qkv[192 * h + 128 : 192 * h + 192]
    b_eff = (b_proj.astype(np.float64) + w_proj.astype(np.float64) @ bv).astype(
        np.float32
    )

    sel = np.zeros((128, 4 * NG), dtype=np.float32)
    esel = np.zeros((NG, 4 * 128), dtype=np.float32)
    for j in range(4):
        for c in range(128):
            sel[c, NG * j + 8 * j + c // GS] = 1.0 / GS
            esel[8 * j + c // GS, 128 * j + c] = 1.0

    cv = np.zeros((128, 12), dtype=np.float32)
    cv[:, 0:4] = b_eff.reshape(4, 128).T
    cv[:, 4:8] = norm_w.reshape(4, 128).T
    cv[:, 8:12] = norm_b.reshape(4, 128).T

    shared = {
        "wqkT": wqkT,
        "wpT": wpT,
        "cvec": cv,
        "sel": sel,
        "esel": esel,
    }
    in_maps = []
    for c in range(N_CORES):
        m = dict(shared)
        m["x2"] = np.ascontiguousarray(x[BPC * c : BPC * (c + 1)])
        in_maps.append(m)
    return in_maps


def _run(in_maps, trace=False):
    nc = _get_nc()
    return run_bass_kernel_spmd(
        nc, in_maps, core_ids=list(range(N_CORES)), trace=trace
    )


def kernel(x, norm_w, norm_b, w_qkv, b_qkv, w_proj, b_proj):
    in_maps = _prep_inputs(x, norm_w, norm_b, w_qkv, b_qkv, w_proj, b_proj)
    res = _run(in_maps)
    out = np.concatenate([r["out"] for r in res.results], axis=0)
    return out.astype(np.float32)
